# revision 1
# baseline (speedup 1.0000x reference)
"""Pointer-network decoder (LSTM + additive attention + argmax masking) on 8 TRN2 cores.

Data-parallel over batch: 512 rows -> 8 cores x 64 rows each; the full
128-step decode loop runs on-chip (tc.For_i), no host round-trips.

Per-core design (B=64 rows, L=128 positions, D=512, E=256):
  - enc_projT resident in SBUF as 4 chunks [128 (d), 8192 (b*128+l)] fp32;
    enc_proj also saved to DRAM in natural row layout for exact rescoring.
  - z = x@Wx + h@Wh on PE in fp16 (weights + activations; fp16 mantissa ==
    tf32-class for these magnitudes, measured numerically free), gates on ACT
    with sigmoid(x) = (tanh(x/2)+1)/2 so tanh/exp share one ACT table set,
    q = h@W2 on PE in float32r (tf32-class, measured 1.5e-4).
  - attention: exact-fp32 DVE broadcast-add (stride-0 AP) of q onto enc_projT,
    ACT tanh -> fp16, v-contraction on PE fp16 with col-tiled [32,512] outputs
    at partition bases 0/32/64/96, full-tile PSUM evacuation + strided
    SBUF->SBUF remap DMA into [64 (b), 128 (l)] layout.
  - tail: mask via accumulated -1e6, reduce_max(negate) -> exp(bias=-max),
    Max8/MaxIndex argmax, then an EXACT fp32 top-2 rescore: gather the two
    candidate enc_proj rows from DRAM (indirect DMA), recompute
    v.tanh(ep+q) in fp32, and pick the true argmax. This removes the
    cheap-path rounding from the (chaotic) argmax trajectory: measured
    absmax vs the jax reference is 0.03 with 6/8.4M elements > 0.01 --
    better than an independent pure-fp32 reimplementation (0.17), since
    fp64-vs-fp32 already flips 2/65536 decisions.
  - next token embedding without gathers: tok = dot(enc_input_row, onehot(idx))
    via a fused DVE multiply-accumulate, then x.T = embT @ onehot(tok) on PE.
"""
import sys

sys.path.insert(0, "/opt/trn_rl_repo")

from contextlib import ExitStack

import numpy as np

import concourse.bass as bass
import concourse.mybir as mybir
import concourse.tile as tile
from concourse import bacc
from concourse.bass import IndirectOffsetOnAxis
from concourse.bass_utils import run_bass_kernel_spmd
from concourse.masks import make_identity

F32 = mybir.dt.float32
F32R = mybir.dt.float32r
I32 = mybir.dt.int32
U32 = mybir.dt.uint32
AX = mybir.AxisListType
OP = mybir.AluOpType
ACTF = mybir.ActivationFunctionType

NCORES = 8
B = 64          # rows per core
L = 128         # positions == decode steps
D = 512
E = 256
V = 1024
SOS = 1
BIG = 1.0e6
NBL = B * L     # 8192 attention columns per core
SL = 2048       # attention column-slice (multiple of 512)
NSTEPS = L


def build(nsteps=NSTEPS, with_biases=False, att_bufs=2, ablate=(), unroll=False):
    nc = bacc.Bacc("TRN2", target_bir_lowering=False, debug=False)

    dec_h0 = nc.dram_tensor("dec_h0", [B, D], F32, kind="ExternalInput")
    dec_c0 = nc.dram_tensor("dec_c0", [B, D], F32, kind="ExternalInput")
    enc_outputs = nc.dram_tensor("enc_outputs", [B, L, D], F32, kind="ExternalInput")
    enc_input = nc.dram_tensor("enc_input", [B, L], I32, kind="ExternalInput")
    emb = nc.dram_tensor("emb", [V, E], F32, kind="ExternalInput")
    Wx = nc.dram_tensor("Wx", [E, 4 * D], F32, kind="ExternalInput")
    Wh = nc.dram_tensor("Wh", [D, 4 * D], F32, kind="ExternalInput")
    bB = nc.dram_tensor("bB", [4 * D], F32, kind="ExternalInput")
    W1 = nc.dram_tensor("W1", [D, D], F32, kind="ExternalInput")
    b1 = nc.dram_tensor("b1", [D], F32, kind="ExternalInput")
    W2 = nc.dram_tensor("W2", [D, D], F32, kind="ExternalInput")
    b2 = nc.dram_tensor("b2", [D], F32, kind="ExternalInput")
    vv = nc.dram_tensor("vv", [D], F32, kind="ExternalInput")
    bv = nc.dram_tensor("bv", [1], F32, kind="ExternalInput")

    out = nc.dram_tensor("out", [NBL, L], F32, kind="ExternalOutput")
    ep_nat = nc.dram_tensor("ep_nat", [NBL, D], F32)  # enc_proj rows for rescoring

    with tile.TileContext(nc) as tc, ExitStack() as ctx:
        consts = ctx.enter_context(tc.tile_pool(name="consts", bufs=1))
        state = ctx.enter_context(tc.tile_pool(name="state", bufs=1))
        big = ctx.enter_context(tc.tile_pool(name="big", bufs=1))

        ident = consts.tile([128, 128], F32)
        make_identity(nc, ident[:])
        identH = consts.tile([B, B], mybir.dt.float16)
        make_identity(nc, identH[:])
        iotaF = consts.tile([B, L], F32)       # value l per column (float)
        iota_b = consts.tile([B, 1], I32)      # value b*L per row
        nc.gpsimd.iota(iota_b[:], pattern=[[0, 1]], base=0, channel_multiplier=L)
        bvS = consts.tile([B, 1], F32)
        nc.sync.dma_start(out=bvS[:], in_=bv[None, 0:1].to_broadcast([B, 1]))
        vS = consts.tile([128, 4, 32], mybir.dt.float16)   # v chunk in col 0 of each 32-col group
        vrep = consts.tile([B, D], F32)        # v replicated per row (rescore dot)
        if with_biases:
            ones_r = consts.tile([1, B], F32R)
            b_r = consts.tile([1, 4 * D], F32R)
            b2_r = consts.tile([1, D], F32R)

        encT = [big.tile([128, NBL], F32, name=f"encT{c}") for c in range(4)]
        emb_sb = big.tile([128, 8, E], mybir.dt.float16)  # emb[vc*128+p, e]
        encinF = consts.tile([B, L], F32)      # enc_input as float
        iotaV = consts.tile([B, 8, L], F32)    # value vc*128+l
        W_sb = big.tile([128, 6, 4 * D], mybir.dt.float16)   # [di-part, chunk, 4D]
        W2_sb = big.tile([128, 4, D], F32R)        # [di-part, di-chunk, do]

        with tc.tile_pool(name="setup", bufs=2) as setup, \
             tc.tile_pool(name="setup_ps", bufs=2, space="PSUM") as setup_ps:
            iotaI = setup.tile([B, L], I32, bufs=1)
            nc.gpsimd.iota(iotaI[:], pattern=[[1, L]], base=0, channel_multiplier=0)
            nc.vector.tensor_copy(iotaF[:], iotaI[:])

            eitmp = setup.tile([B, L], I32, bufs=1)
            nc.sync.dma_start(out=eitmp[:], in_=enc_input[:, :])
            nc.vector.tensor_copy(encinF[:], eitmp[:])
            ivtmp = setup.tile([B, 8 * L], I32, bufs=1)
            nc.gpsimd.iota(ivtmp[:], pattern=[[1, 8 * L]], base=0, channel_multiplier=0)
            nc.vector.tensor_copy(iotaV[:].rearrange("b c l -> b (c l)"), ivtmp[:])
            for vc in range(8):
                emtmp = setup.tile([128, E], F32, name="emtmp", bufs=1)
                nc.sync.dma_start(out=emtmp[:], in_=emb[128 * vc:128 * (vc + 1), :])
                nc.vector.tensor_copy(emb_sb[:, vc, :], emtmp[:])

            nc.sync.dma_start(out=vrep[:], in_=vv[None, :].to_broadcast([B, D]))
            vtmp = setup.tile([128, 4, 32], F32, bufs=1)
            nc.vector.memset(vtmp[:], 0.0)
            for c in range(4):
                nc.sync.dma_start(
                    out=vtmp[:, c, 0:1], in_=vv[128 * c:128 * (c + 1), None]
                )
            nc.vector.tensor_copy(vS[:], vtmp[:])

            for c in range(2):
                wt = setup.tile([128, 4 * D], F32, name="wtmp", bufs=1)
                nc.sync.dma_start(out=wt[:], in_=Wx[128 * c:128 * (c + 1), :])
                nc.vector.tensor_copy(W_sb[:, c, :], wt[:])
            for c in range(4):
                wt = setup.tile([128, 4 * D], F32, name="wtmp", bufs=1)
                nc.sync.dma_start(out=wt[:], in_=Wh[128 * c:128 * (c + 1), :])
                nc.vector.tensor_copy(W_sb[:, c + 2, :], wt[:])
            for c in range(4):
                wt2 = setup.tile([128, D], F32, name="w2tmp", bufs=1)
                nc.sync.dma_start(out=wt2[:], in_=W2[128 * c:128 * (c + 1), :])
                nc.vector.tensor_copy(W2_sb[:, c, :], wt2[:])

            if with_biases:
                ot = setup.tile([1, B], F32, bufs=1)
                nc.vector.memset(ot[:], 1.0)
                nc.vector.tensor_copy(ones_r[:], ot[:])
                bt = setup.tile([1, 4 * D], F32, bufs=1)
                nc.sync.dma_start(out=bt[:], in_=bB[None, :])
                nc.vector.tensor_copy(b_r[:], bt[:])
                b2t = setup.tile([1, D], F32, bufs=1)
                nc.sync.dma_start(out=b2t[:], in_=b2[None, :])
                nc.vector.tensor_copy(b2_r[:], b2t[:])

        with tc.tile_pool(name="setup2", bufs=2) as setup, \
             tc.tile_pool(name="setup2_ps", bufs=2, space="PSUM") as setup_ps:
            # ---- enc_projT build ----
            W1_sb = setup.tile([128, 4, D], F32, bufs=1)
            for c in range(4):
                nc.sync.dma_start(out=W1_sb[:, c, :], in_=W1[128 * c:128 * (c + 1), :])
            b1rep = setup.tile([128, D], F32, bufs=1)
            nc.sync.dma_start(out=b1rep[:], in_=b1[None, :].to_broadcast([128, D]))
            b1S = setup.tile([128, 4], F32, bufs=1)
            for c in range(4):
                nc.sync.dma_start(
                    out=b1S[:, c:c + 1], in_=b1[128 * c:128 * (c + 1), None]
                )

            ENCr = enc_outputs[:, :, :].rearrange("b l d -> (b l) d")
            for rs in range(NBL // 128):
                encs = setup.tile([128, D], F32, name="encs")
                nc.sync.dma_start(out=encs[:], in_=ENCr[128 * rs:128 * (rs + 1), :])
                eot = []
                for di in range(4):
                    tp = setup_ps.tile([128, 128], F32, name="tp_ps")
                    nc.tensor.transpose(tp[:], encs[:, 128 * di:128 * (di + 1)], ident[:])
                    es = setup.tile([128, 128], F32, name=f"eot{di}", bufs=1)
                    nc.vector.tensor_copy(es[:], tp[:])
                    eot.append(es)
                for do in range(4):
                    ps = setup_ps.tile([128, 128], F32, name="mm_ps")
                    for di in range(4):
                        nc.tensor.matmul(
                            ps[:],
                            lhsT=W1_sb[:, di, 128 * do:128 * (do + 1)],
                            rhs=eot[di][:],
                            start=(di == 0),
                            stop=(di == 3),
                        )
                    nc.vector.tensor_copy(encT[do][:, 128 * rs:128 * (rs + 1)], ps[:])
                # natural-orientation rows for the exact rescore path
                nps = setup_ps.tile([128, D], F32, name="nat_ps")
                for di in range(4):
                    nc.tensor.matmul(
                        nps[:],
                        lhsT=eot[di][:],
                        rhs=W1_sb[:, di, :],
                        start=(di == 0),
                        stop=(di == 3),
                    )
                nat = setup.tile([128, D], F32, name="nat", bufs=2)
                nc.vector.tensor_tensor(
                    out=nat[:], in0=nps[:], in1=b1rep[:], op=OP.add,
                )
                nc.sync.dma_start(
                    out=ep_nat[128 * rs:128 * (rs + 1), :], in_=nat[:]
                )
            for c in range(4):
                nc.vector.tensor_scalar(
                    out=encT[c][:], in0=encT[c][:], scalar1=b1S[:, c:c + 1],
                    scalar2=None, op0=OP.add,
                )

        # ---------- mutable state ----------
        c_sb = state.tile([B, D], F32)
        nc.sync.dma_start(out=c_sb[:], in_=dec_c0[:, :])
        maskBig = state.tile([B, L], F32)
        nc.vector.memset(maskBig[:], 0.0)
        hTS = state.tile([128, 4 * B], mybir.dt.float16)   # h(t-1).T chunks: [di, dc*64+b]
        hTSr = state.tile([128, 4 * B], F32R)   # same, f32r for the q matmul
        xTS = state.tile([128, 2 * B], mybir.dt.float16)   # x(t).T chunks

        with tc.tile_pool(name="init", bufs=1) as init_pool, \
             tc.tile_pool(name="init_ps", bufs=1, space="PSUM") as init_ps:
            h0 = init_pool.tile([B, D], F32)
            nc.sync.dma_start(out=h0[:], in_=dec_h0[:, :])
            htp = init_ps.tile([128, 4 * B], F32)
            for k in range(4):
                nc.tensor.transpose(
                    htp[:, B * k:B * (k + 1)], h0[:, 128 * k:128 * (k + 1)],
                    ident[:B, :B],
                )
            nc.vector.tensor_copy(hTS[:], htp[:])
            nc.vector.tensor_copy(hTSr[:], htp[:])
            x0e = init_pool.tile([128, 2], F32)
            for k in range(2):
                nc.sync.dma_start(
                    out=x0e[:, k:k + 1],
                    in_=bass.AP(emb, SOS * E + 128 * k, [[1, 128], [1, 1]]),
                )
            nc.vector.tensor_copy(
                out=xTS[:].rearrange("p (k b) -> p k b", k=2),
                in_=x0e[:, :, None].to_broadcast([128, 2, B]),
            )

        # ---------- decode loop ----------
        lp = ctx.enter_context(tc.tile_pool(name="lp", bufs=1))
        att = ctx.enter_context(tc.tile_pool(name="att", bufs=att_bufs))
        ps_z = ctx.enter_context(tc.tile_pool(name="ps_z", bufs=1, space="PSUM"))
        ps_s = ctx.enter_context(tc.tile_pool(name="ps_s", bufs=1, space="PSUM"))

        offs = state.tile([B, 1], I32)
        nc.vector.tensor_copy(offs[:], iota_b[:])

        from contextlib import nullcontext

        def loop_ctx():
            if unroll:
                return nullcontext(enumerate(range(nsteps)))
            return tc.For_i(0, nsteps, 1)

        steps = range(nsteps) if unroll else [None]
        for t_step in steps:
          with (nullcontext() if unroll else tc.For_i(0, nsteps, 1)) as iv:
              # ---- z = x@Wx + h@Wh (+ b) : [B, 2048] psum ----
              skip_lstm = "nolstm" in ablate
              z_ps = ps_z.tile([B, 4 * D], F32, tag="zps")
              for ns in range(0 if skip_lstm else 4):
                  nsl = slice(512 * ns, 512 * (ns + 1))
                  for kc in range(6):
                      lhs = xTS[:, B * kc:B * (kc + 1)] if kc < 2 else \
                          hTS[:, B * (kc - 2):B * (kc - 1)]
                      last = (kc == 5) and not with_biases
                      nc.tensor.matmul(
                          z_ps[:, nsl], lhsT=lhs, rhs=W_sb[:, kc, nsl],
                          start=(kc == 0), stop=last,
                      )
                  if with_biases:
                      nc.tensor.matmul(
                          z_ps[:, nsl], lhsT=ones_r[:], rhs=b_r[:, nsl],
                          start=False, stop=True,
                      )

              # ---- gates (sigmoid(x) = (tanh(x/2)+1)/2) ----
              sif = lp.tile([B, 2 * D], F32)
              so = lp.tile([B, D], F32)
              gt = lp.tile([B, D], F32)
              if skip_lstm:
                  nc.vector.memset(sif[:], 0.5)
                  nc.vector.memset(so[:], 0.5)
              else:
                  nc.scalar.activation(sif[:], z_ps[:, 0:2 * D], ACTF.Tanh, scale=0.5)
                  nc.vector.tensor_scalar(
                      out=sif[:], in0=sif[:], scalar1=1.0, scalar2=0.5,
                      op0=OP.add, op1=OP.mult,
                  )
                  nc.scalar.activation(so[:], z_ps[:, 3 * D:4 * D], ACTF.Tanh, scale=0.5)
                  nc.vector.tensor_scalar(
                      out=so[:], in0=so[:], scalar1=1.0, scalar2=0.5,
                      op0=OP.add, op1=OP.mult,
                  )
                  nc.scalar.activation(gt[:], z_ps[:, 2 * D:3 * D], ACTF.Tanh)
                  # c = f*c + i*tanh(g);  h = o*tanh(c)   (in-place reuse)
                  nc.vector.tensor_tensor(out=c_sb[:], in0=sif[:, D:2 * D], in1=c_sb[:], op=OP.mult)
                  nc.vector.tensor_tensor(out=gt[:], in0=sif[:, 0:D], in1=gt[:], op=OP.mult)
                  nc.vector.tensor_tensor(out=c_sb[:], in0=c_sb[:], in1=gt[:], op=OP.add)
                  nc.scalar.activation(gt[:], c_sb[:], ACTF.Tanh)
                  nc.vector.tensor_tensor(out=so[:], in0=so[:], in1=gt[:], op=OP.mult)
              h_sb = so  # h(t)

              # ---- hT (for next z and q) ----
              trT = ps_s.tile([128, 4 * B], F32, tag="trT")
              for k in range(4):
                  nc.tensor.transpose(
                      trT[:, B * k:B * (k + 1)], h_sb[:, 128 * k:128 * (k + 1)],
                      ident[:B, :B],
                  )
              nc.vector.tensor_copy(hTS[:], trT[:])
              nc.vector.tensor_copy(hTSr[:], trT[:])

              # ---- q = h@W2 (+ b2): qB -> qTS [128, 4*B] fp32 ----
              qb_ps = ps_z.tile([B, D], F32, tag="zps")
              for di in range(4):
                  last = (di == 3) and not with_biases
                  nc.tensor.matmul(
                      qb_ps[:], lhsT=hTSr[:, B * di:B * (di + 1)], rhs=W2_sb[:, di, :],
                      start=(di == 0), stop=last,
                  )
              if with_biases:
                  nc.tensor.matmul(
                      qb_ps[:], lhsT=ones_r[:], rhs=b2_r[:], start=False, stop=True,
                  )
              qB = sif[:, 0:D]  # sif is dead; reuse its slot
              nc.vector.tensor_copy(qB, qb_ps[:])
              for k in range(4):
                  nc.tensor.transpose(
                      trT[:, B * k:B * (k + 1)], qB[:, 128 * k:128 * (k + 1)],
                      ident[:B, :B],
                  )
              qTS = lp.tile([128, 4 * B], F32)
              nc.vector.tensor_copy(qTS[:], trT[:])

              # ---- attention: tanh(encT + qT) . v ----
              lg64 = lp.tile([B, L], F32)
              nb = SL // L
              if "noatt" in ablate:
                  nc.vector.memset(lg64[:], 0.0)
              for s in range(() if "noatt" in ablate else range(NBL // SL)) if False else (
                  [] if "noatt" in ablate else range(NBL // SL)):
                  lgP = ps_s.tile([128, 512], F32, tag="lgP", bufs=2)
                  for c in range(4):
                      T = att.tile([128, SL], mybir.dt.float16, tag="attT")
                      nc.vector.tensor_tensor(
                          out=T[:].rearrange("p (b l) -> p b l", b=nb),
                          in0=encT[c][:, SL * s:SL * (s + 1)].rearrange(
                              "p (b l) -> p b l", b=nb),
                          in1=qTS[:, B * c + nb * s:B * c + nb * (s + 1), None]
                              .to_broadcast([128, nb, L]),
                          op=OP.add,
                      )
                      nc.scalar.activation(T[:], T[:], ACTF.Tanh)
                      for j2 in range(SL // 512):
                          nc.tensor.matmul(
                              lgP[32 * j2:32 * (j2 + 1), :],
                              lhsT=vS[:, c, :],
                              rhs=T[:, 512 * j2:512 * (j2 + 1)],
                              start=(c == 0), stop=(c == 3),
                              tile_position=(0, 32 * j2),
                          )
                  # harvest: psum row 32*j2 -> lg64 rows [16s+4*j2, +4)
                  lgH = lp.tile([128, 512], F32, tag="lgH")
                  nc.vector.tensor_copy(out=lgH[:], in_=lgP[:])
                  for j2 in range(SL // 512):
                      nc.sync.dma_start(
                          out=lg64[16 * s + 4 * j2:16 * s + 4 * j2 + 4, :],
                          in_=lgH[32 * j2:32 * j2 + 1, :].rearrange(
                              "p (k l) -> p k l", k=4),
                      )

              # ---- softmax / argmax tail in [B, L] layout ----
              nc.vector.scalar_tensor_tensor(
                  out=lg64[:], in0=lg64[:], scalar=bvS[:], in1=maskBig[:],
                  op0=OP.add, op1=OP.subtract,
              )
              nmx = lp.tile([B, 1], F32)
              nc.vector.reduce_max(out=nmx[:], in_=lg64[:], axis=AX.X, negate=True)
              pex = lp.tile([B, L], F32)
              nc.scalar.activation(pex[:], lg64[:], ACTF.Exp, bias=nmx[:], scale=1.0)
              ssum = lp.tile([B, 1], F32)
              nc.vector.reduce_sum(out=ssum[:], in_=pex[:], axis=AX.X)
              rsum = lp.tile([B, 1], F32)
              nc.vector.reciprocal(rsum[:], ssum[:])
              nc.vector.tensor_scalar(
                  out=pex[:], in0=pex[:], scalar1=rsum[:], scalar2=None, op0=OP.mult,
              )
              if "noscat" not in ablate:
                  nc.gpsimd.indirect_dma_start(
                      out=out[:, :],
                      out_offset=IndirectOffsetOnAxis(ap=offs[:, :1], axis=0),
                      in_=pex[:], in_offset=None,
                  )
                  nc.vector.tensor_scalar(
                      out=offs[:], in0=offs[:], scalar1=1, scalar2=None, op0=OP.add,
                  )

              mx8 = lp.tile([B, 8], F32)
              ix8 = lp.tile([B, 8], U32)
              nc.vector.max(mx8[:], lg64[:])
              nc.vector.max_index(ix8[:], mx8[:], lg64[:])
              # exact fp32 rescore of the top-2 candidates
              if "norescue" not in ablate:
                  resc = lp.tile([B, 2], F32)
                  for k in range(2):
                      rk = lp.tile([B, 1], I32, name=f"rk{k}")
                      nc.vector.tensor_tensor(
                          out=rk[:], in0=ix8[:, k:k + 1].bitcast(I32),
                          in1=iota_b[:], op=OP.add,
                      )
                      epc = lp.tile([B, D], F32, name=f"epc{k}")
                      nc.gpsimd.indirect_dma_start(
                          out=epc[:], out_offset=None, in_=ep_nat[:, :],
                          in_offset=IndirectOffsetOnAxis(ap=rk[:, :1], axis=0),
                      )
                      nc.vector.tensor_tensor(
                          out=epc[:], in0=epc[:], in1=qB, op=OP.add,
                      )
                      nc.scalar.activation(epc[:], epc[:], ACTF.Tanh)
                      nc.vector.scalar_tensor_tensor(
                          out=epc[:], in0=epc[:], scalar=1.0, in1=vrep[:],
                          op0=OP.mult, op1=OP.mult, accum_out=resc[:, k:k + 1],
                      )
                  # pick = 1 if cand1 strictly better
                  pick = lp.tile([B, 1], F32)
                  nc.vector.tensor_tensor(
                      out=pick[:], in0=resc[:, 1:2], in1=resc[:, 0:1],
                      op=OP.is_gt,
                  )
                  idxF = lp.tile([B, 1], F32)
                  ix0 = lp.tile([B, 2], F32)
                  nc.vector.tensor_copy(ix0[:], ix8[:, 0:2])
                  nc.vector.scalar_tensor_tensor(
                      out=idxF[:], in0=ix0[:, 1:2], scalar=1.0, in1=ix0[:, 0:1],
                      op0=OP.mult, op1=OP.subtract,
                  )
                  # idxF currently = idx1 - idx0 ; idx = idx0 + pick*(idx1-idx0)
                  nc.vector.scalar_tensor_tensor(
                      out=idxF[:], in0=idxF[:], scalar=pick[:], in1=ix0[:, 0:1],
                      op0=OP.mult, op1=OP.add,
                  )
              else:
                  idxF = lp.tile([B, 1], F32)
                  nc.vector.tensor_copy(idxF[:], ix8[:, 0:1])
              eqf = lp.tile([B, L], F32)
              nc.vector.tensor_scalar(
                  out=eqf[:], in0=iotaF[:], scalar1=idxF[:], scalar2=None,
                  op0=OP.is_equal,
              )
              nc.vector.scalar_tensor_tensor(
                  out=maskBig[:], in0=eqf[:], scalar=BIG, in1=maskBig[:],
                  op0=OP.mult, op1=OP.add,
              )

              if "noxpath" not in ablate:
                  # tok = enc_input[b, idx[b]] via dot with the idx one-hot
                  tokF = lp.tile([B, 1], F32)
                  nc.vector.scalar_tensor_tensor(
                      out=eqf[:], in0=encinF[:], scalar=1.0, in1=eqf[:],
                      op0=OP.mult, op1=OP.mult, accum_out=tokF[:],
                  )
                  # one-hot of tok over the vocab, transposed to [v-part, b]
                  eqV = lp.tile([B, 8, L], mybir.dt.float16)
                  nc.vector.tensor_scalar(
                      out=eqV[:], in0=iotaV[:], scalar1=tokF[:], scalar2=None,
                      op0=OP.is_equal,
                  )
                  ohT = ps_s.tile([128, 8, B], mybir.dt.float16, tag="ohT")
                  for vc in range(8):
                      nc.tensor.transpose(ohT[:, vc, :], eqV[:, vc, :], identH[:])
                  ohS = lp.tile([128, 8, B], mybir.dt.float16)
                  nc.vector.tensor_copy(ohS[:], ohT[:])
                  # xT = embT @ onehot : accumulate over v-chunks
                  for ec in range(2):
                      for vc in range(8):
                          nc.tensor.matmul(
                              trT[:, B * ec:B * (ec + 1)],
                              lhsT=emb_sb[:, vc, 128 * ec:128 * (ec + 1)],
                              rhs=ohS[:, vc, :],
                              start=(vc == 0), stop=(vc == 7),
                          )
                  nc.vector.tensor_copy(xTS[:], trT[:, 0:2 * B])

    nc.finalize()
    return nc


_CACHED = {}


def _get_nc(key):
    if key not in _CACHED:
        _CACHED[key] = build(*key)
    return _CACHED[key]


ABLATE = ()


def kernel(dec_h0, dec_c0, enc_outputs, enc_input, emb, Wx, Wh, b,
           W1, b1, W2, b2, v, bv, nsteps=NSTEPS, trace=False):
    dec_h0 = np.ascontiguousarray(np.asarray(dec_h0, dtype=np.float32))
    dec_c0 = np.ascontiguousarray(np.asarray(dec_c0, dtype=np.float32))
    enc_outputs = np.ascontiguousarray(np.asarray(enc_outputs, dtype=np.float32))
    enc_input = np.ascontiguousarray(np.asarray(enc_input, dtype=np.int32))
    shared = dict(
        emb=np.ascontiguousarray(np.asarray(emb, np.float32)),
        Wx=np.ascontiguousarray(np.asarray(Wx, np.float32)),
        Wh=np.ascontiguousarray(np.asarray(Wh, np.float32)),
        bB=np.ascontiguousarray(np.asarray(b, np.float32)),
        W1=np.ascontiguousarray(np.asarray(W1, np.float32)),
        b1=np.ascontiguousarray(np.asarray(b1, np.float32)),
        W2=np.ascontiguousarray(np.asarray(W2, np.float32)),
        b2=np.ascontiguousarray(np.asarray(b2, np.float32)),
        vv=np.ascontiguousarray(np.asarray(v, np.float32)),
        bv=np.full((1,), float(np.asarray(bv)), np.float32),
    )
    with_biases = bool(
        np.any(shared["bB"]) or np.any(shared["b2"]) or np.any(shared["bv"])
    )
    BT = dec_h0.shape[0]
    assert BT == NCORES * B, (BT, NCORES * B)

    in_maps = []
    for c in range(NCORES):
        sl = slice(B * c, B * (c + 1))
        m = dict(shared)
        m.update(
            dec_h0=dec_h0[sl], dec_c0=dec_c0[sl],
            enc_outputs=enc_outputs[sl], enc_input=enc_input[sl],
        )
        in_maps.append(m)

    nc = _get_nc((nsteps, with_biases, 1 if with_biases else 2, tuple(ABLATE)))
    res = run_bass_kernel_spmd(
        nc, in_maps, core_ids=list(range(NCORES)), trace=trace,
    )
    outs = [
        res.results[c]["out"].reshape(B, L, L)[:, :nsteps, :]
        for c in range(NCORES)
    ]
    full = np.concatenate(outs, axis=0)
    kernel.last_results = res
    return full


if __name__ == "__main__":
    d = np.load("/root/problem/ref_data.npz")
    out = kernel(
        d["dec_h0"], d["dec_c0"], d["enc_outputs"], d["enc_input"], d["emb"],
        d["Wx"], d["Wh"], d["b"], d["W1"], d["b1"], d["W2"], d["b2"],
        d["v"], d["bv"], nsteps=int(sys.argv[1]) if len(sys.argv) > 1 else NSTEPS,
    )
    print("out", out.shape, out.dtype)

